# revision 2
# baseline (speedup 1.0000x reference)
"""Pairwise squared L2 distance (retrieval KNN) on 8 TRN2 NeuronCores.

dist[i, j] = ||x_i||^2 + ||y_j||^2 - 2 * <x_i, y_j>

Sharding: rows of x are split across the 8 cores (data-parallel over n);
y is replicated. Each core computes a [1024, 8192] slab of the distance
matrix.

The kernel is HBM-write-bound (32 MiB of fp32 output per core), so the
math is trimmed to the minimum byte/engine budget that stays inside the
2e-2 relative-error gate (~5e-4 measured):
 - cross term: ONE fp16 matmul per tile, with x pre-scaled by -2 on the
   host (exact power-of-two scale), fp32 PSUM accumulate;
 - + y_sq: folded into the same PSUM group as a contraction-1 matmul
   (ones[1,128] x ysq16[1,NT]) -- no DVE pass, no broadcast tile;
 - + x_sq: per-partition bias on the ScalarE PSUM->SBUF copy.
Per-block chain is PE -> ACT -> DMA store; loads are ordered so the
first block's deps (~0.8 MiB) land before the bulk of y streams in.
"""

import numpy as np

import concourse.bass as bass
import concourse.mybir as mybir
import concourse.tile as tile
from concourse import bacc
from concourse.bass import ts
from concourse.bass_utils import run_bass_kernel_spmd

N, M, D = 8192, 8192, 128
NCORES = 8
SLAB = N // NCORES  # 1024 rows of x per core
P = 128  # partitions / m-chunk height
MCH = SLAB // P  # 8 m-chunks per core
NT = 512  # matmul free-dim tile (one fp32 PSUM bank)
GW = 4  # n-chunks per PSUM group (4 banks = 8 KiB/partition)
GCOLS = GW * NT  # 2048
NG = M // GCOLS  # 4 column groups

_f32 = mybir.dt.float32
_f16 = mybir.dt.float16
_IDENT = mybir.ActivationFunctionType.Identity

_compiled_nc = None


def _build():
    """Build + compile the single-core Bass program (SPMD across 8 cores)."""
    nc = bacc.Bacc(
        "TRN2",
        target_bir_lowering=False,
        debug=False,
        enable_asserts=False,
        num_devices=NCORES,
    )
    xs2 = nc.dram_tensor("xs2", [D, SLAB], _f16, kind="ExternalInput").ap()
    yh = nc.dram_tensor("yh", [D, M], _f16, kind="ExternalInput").ap()
    ysq = nc.dram_tensor("ysq", [1, M], _f16, kind="ExternalInput").ap()
    xsq = nc.dram_tensor("xsq", [P, MCH], _f32, kind="ExternalInput").ap()
    dist = nc.dram_tensor("dist", [SLAB, M], _f32, kind="ExternalOutput").ap()

    with tile.TileContext(nc) as tc:
        with (
            tc.tile_pool(name="consts", bufs=1) as cpool,
            tc.tile_pool(name="psum", bufs=2, space="PSUM") as pspool,
            tc.tile_pool(name="obuf", bufs=6) as opool,
        ):
            # Critical-path loads first: everything block (g=0, mc=0) needs.
            yh_sb = cpool.tile([D, M], _f16)
            nc.sync.dma_start(yh_sb[:, ts(0, GCOLS)], yh[:, ts(0, GCOLS)])
            xs2_sb = cpool.tile([D, SLAB], _f16)
            nc.sync.dma_start(xs2_sb[:], xs2[:])
            ysq_sb = cpool.tile([1, M], _f16)
            nc.sync.dma_start(ysq_sb[:], ysq[:])
            xsq_sb = cpool.tile([P, MCH], _f32)
            nc.sync.dma_start(xsq_sb[:], xsq[:])
            ones = cpool.tile([1, P], _f16)
            nc.vector.memset(ones[:], 1.0)
            # Bulk of y streams in behind the first group's work.
            for g in range(1, NG):
                nc.sync.dma_start(yh_sb[:, ts(g, GCOLS)], yh[:, ts(g, GCOLS)])

            def emit_block(mc, g):
                """One [128, GCOLS] output block -> dist[mc*128:, g*GCOLS:]."""
                xw = xs2_sb[:, ts(mc, P)]
                ps = pspool.tile([P, GCOLS], _f32, tag="ps")
                # xw held stationary for GW matmuls, then the ones row.
                for jj in range(GW):
                    nc.tensor.matmul(
                        ps[:, ts(jj, NT)],
                        xw,
                        yh_sb[:, ts(g * GW + jj, NT)],
                        start=True,
                        stop=False,
                    )
                for jj in range(GW):
                    nc.tensor.matmul(
                        ps[:, ts(jj, NT)],
                        ones[:],
                        ysq_sb[:, ts(g * GW + jj, NT)],
                        start=False,
                        stop=True,
                    )
                # Epilogue: out = psum + x_sq (per-partition bias) on ScalarE.
                ot = opool.tile([P, GCOLS], _f32, tag="ot")
                nc.scalar.activation(
                    ot[:],
                    ps[:],
                    _IDENT,
                    bias=xsq_sb[:, mc : mc + 1],
                    scale=1.0,
                )
                nc.sync.dma_start(dist[ts(mc, P), ts(g, GCOLS)], ot[:])

            for g in range(NG):
                for mc in range(MCH):
                    emit_block(mc, g)

    nc.compile()
    return nc


def _get_nc():
    global _compiled_nc
    if _compiled_nc is None:
        _compiled_nc = _build()
    return _compiled_nc


def make_in_maps(x: np.ndarray, y: np.ndarray) -> list[dict[str, np.ndarray]]:
    x = np.asarray(x, dtype=np.float32)
    y = np.asarray(y, dtype=np.float32)
    x_sq = np.sum(x * x, axis=1, dtype=np.float32)
    y_sq = np.sum(y * y, axis=1, dtype=np.float32)

    xs2t = np.ascontiguousarray((-2.0 * x).T.astype(np.float16))  # [D, N]
    yt_hi = np.ascontiguousarray(y.T.astype(np.float16))  # [D, M]
    ysq_in = np.ascontiguousarray(y_sq.reshape(1, M).astype(np.float16))

    in_maps = []
    for c in range(NCORES):
        sl = slice(c * SLAB, (c + 1) * SLAB)
        # [P, MCH]: column mc holds x_sq for rows mc*128..mc*128+127
        xsq_in = np.ascontiguousarray(x_sq[sl].reshape(MCH, P).T)
        in_maps.append(
            {
                "xs2": np.ascontiguousarray(xs2t[:, sl]),
                "yh": yt_hi,
                "ysq": ysq_in,
                "xsq": xsq_in,
            }
        )
    return in_maps


def kernel(x: np.ndarray, y: np.ndarray, **run_kwargs) -> np.ndarray:
    nc = _get_nc()
    in_maps = make_in_maps(x, y)
    res = run_bass_kernel_spmd(nc, in_maps, core_ids=list(range(NCORES)), **run_kwargs)
    out = np.concatenate([res.results[c]["dist"] for c in range(NCORES)], axis=0)
    if run_kwargs:
        kernel.last_results = res
    return out


# revision 3
# speedup vs baseline: 1.0945x; 1.0945x over previous
"""Pairwise squared L2 distance (retrieval KNN) on 8 TRN2 NeuronCores.

dist[i, j] = ||x_i||^2 + ||y_j||^2 - 2 * <x_i, y_j>

Sharding: rows of x are split across the 8 cores (data-parallel over n);
y is replicated. Each core computes a [1024, 8192] slab of the distance
matrix.

The kernel is HBM-write-bound (32 MiB of fp32 output per core), so the
math is trimmed to the minimum byte budget that stays inside the 2e-2
relative-error gate (~2e-4 measured): the cross term is ONE fp16 matmul
per tile with x pre-scaled by -2 on the host (exact power-of-two scale),
fp32 PSUM accumulate.  Norms ride the epilogue: ScalarE computes
psum + x_sq (per-partition bias), VectorE adds a broadcast y_sq tile
(built once on-chip by gpsimd partition_broadcast, exact fp32), and
1 MiB stores stream the result out.  Loads are ordered so the first
block's deps (~0.8 MiB) land before the bulk of y streams in.
"""

import numpy as np

import concourse.bass as bass
import concourse.mybir as mybir
import concourse.tile as tile
from concourse import bacc
from concourse.bass import ts
from concourse.bass_utils import run_bass_kernel_spmd

N, M, D = 8192, 8192, 128
NCORES = 8
SLAB = N // NCORES  # 1024 rows of x per core
P = 128  # partitions / m-chunk height
MCH = SLAB // P  # 8 m-chunks per core
NT = 512  # matmul free-dim tile (one fp32 PSUM bank)
GW = 4  # n-chunks per PSUM group (4 banks = 8 KiB/partition)
GCOLS = GW * NT  # 2048
NG = M // GCOLS  # 4 column groups

_f32 = mybir.dt.float32
_f16 = mybir.dt.float16
_IDENT = mybir.ActivationFunctionType.Identity

_compiled_nc = None


def _build():
    """Build + compile the single-core Bass program (SPMD across 8 cores)."""
    nc = bacc.Bacc(
        "TRN2",
        target_bir_lowering=False,
        debug=False,
        enable_asserts=False,
        num_devices=NCORES,
    )
    xs2 = nc.dram_tensor("xs2", [D, SLAB], _f16, kind="ExternalInput").ap()
    yh = nc.dram_tensor("yh", [D, M], _f16, kind="ExternalInput").ap()
    ysq = nc.dram_tensor("ysq", [1, M], _f32, kind="ExternalInput").ap()
    xsq = nc.dram_tensor("xsq", [P, MCH], _f32, kind="ExternalInput").ap()
    dist = nc.dram_tensor("dist", [SLAB, M], _f32, kind="ExternalOutput").ap()

    with tile.TileContext(nc) as tc:
        with (
            tc.tile_pool(name="consts", bufs=1) as cpool,
            tc.tile_pool(name="psum", bufs=2, space="PSUM") as pspool,
            tc.tile_pool(name="abuf", bufs=4) as apool,
            tc.tile_pool(name="obuf", bufs=6) as opool,
        ):
            # Critical-path loads first: everything block (g=0, mc=0) needs.
            yh_sb = cpool.tile([D, M], _f16)
            nc.sync.dma_start(yh_sb[:, ts(0, GCOLS)], yh[:, ts(0, GCOLS)])
            xs2_sb = cpool.tile([D, SLAB], _f16)
            nc.sync.dma_start(xs2_sb[:], xs2[:])
            ysq_row = cpool.tile([1, M], _f32)
            nc.sync.dma_start(ysq_row[:], ysq[:])
            xsq_sb = cpool.tile([P, MCH], _f32)
            nc.sync.dma_start(xsq_sb[:], xsq[:])
            # Bulk of y streams in behind the first group's work.
            for g in range(1, NG):
                nc.sync.dma_start(yh_sb[:, ts(g, GCOLS)], yh[:, ts(g, GCOLS)])

            # ysq_b[p, j] = y_sq[j], exact fp32, built on the otherwise-idle
            # GpSimd engine in group-sized chunks.
            ysq_b = cpool.tile([P, M], _f32)
            for g in range(NG):
                nc.gpsimd.partition_broadcast(
                    ysq_b[:, ts(g, GCOLS)], ysq_row[0:1, ts(g, GCOLS)]
                )

            def emit_block(mc, g):
                """One [128, GCOLS] output block -> dist[mc*128:, g*GCOLS:]."""
                xw = xs2_sb[:, ts(mc, P)]
                ps = pspool.tile([P, GCOLS], _f32, tag="ps")
                for jj in range(GW):
                    nc.tensor.matmul(
                        ps[:, ts(jj, NT)],
                        xw,
                        yh_sb[:, ts(g * GW + jj, NT)],
                        start=True,
                        stop=True,
                    )
                # Epilogue: a = psum + x_sq (ACT), out = a + y_sq (DVE).
                a = apool.tile([P, GCOLS], _f32, tag="a")
                nc.scalar.activation(
                    a[:],
                    ps[:],
                    _IDENT,
                    bias=xsq_sb[:, mc : mc + 1],
                    scale=1.0,
                )
                ot = opool.tile([P, GCOLS], _f32, tag="ot")
                nc.vector.tensor_add(
                    ot[:], a[:], ysq_b[:, ts(g, GCOLS)]
                )
                nc.sync.dma_start(dist[ts(mc, P), ts(g, GCOLS)], ot[:])

            for g in range(NG):
                for mc in range(MCH):
                    emit_block(mc, g)

    nc.compile()
    return nc


def _get_nc():
    global _compiled_nc
    if _compiled_nc is None:
        _compiled_nc = _build()
    return _compiled_nc


def make_in_maps(x: np.ndarray, y: np.ndarray) -> list[dict[str, np.ndarray]]:
    x = np.asarray(x, dtype=np.float32)
    y = np.asarray(y, dtype=np.float32)
    x_sq = np.sum(x * x, axis=1, dtype=np.float32)
    y_sq = np.sum(y * y, axis=1, dtype=np.float32)

    xs2t = np.ascontiguousarray((-2.0 * x).T.astype(np.float16))  # [D, N]
    yt_hi = np.ascontiguousarray(y.T.astype(np.float16))  # [D, M]
    ysq_in = np.ascontiguousarray(y_sq.reshape(1, M))

    in_maps = []
    for c in range(NCORES):
        sl = slice(c * SLAB, (c + 1) * SLAB)
        # [P, MCH]: column mc holds x_sq for rows mc*128..mc*128+127
        xsq_in = np.ascontiguousarray(x_sq[sl].reshape(MCH, P).T)
        in_maps.append(
            {
                "xs2": np.ascontiguousarray(xs2t[:, sl]),
                "yh": yt_hi,
                "ysq": ysq_in,
                "xsq": xsq_in,
            }
        )
    return in_maps


def kernel(x: np.ndarray, y: np.ndarray, **run_kwargs) -> np.ndarray:
    nc = _get_nc()
    in_maps = make_in_maps(x, y)
    res = run_bass_kernel_spmd(nc, in_maps, core_ids=list(range(NCORES)), **run_kwargs)
    out = np.concatenate([res.results[c]["dist"] for c in range(NCORES)], axis=0)
    if run_kwargs:
        kernel.last_results = res
    return out


# revision 4
# speedup vs baseline: 1.2136x; 1.1089x over previous
"""Pairwise squared L2 distance (retrieval KNN) on 8 TRN2 NeuronCores.

dist[i, j] = ||x_i||^2 + ||y_j||^2 - 2 * <x_i, y_j>

Sharding: rows of x are split across the 8 cores (data-parallel over n);
y is replicated. Each core computes a [1024, 8192] slab of the distance
matrix.

HBM-write-bound kernel (32 MiB fp32 out per core).  Two hardware facts
shape the schedule:
 1. TRN2 clock governor: the chip only holds its fast state while the
    PE runs a dense matmul stream (~90%+ duty); a lean 4-matmul block
    lets everything downclock ~2x (measured).  So each [128, 512] tile
    gets THREE matmuls -- xh@y + xl@y (fp16 hi/lo split of -2x, extra
    precision for free) and a K=2 fold (ones2 @ [ysq_hi; ysq_lo]) that
    adds y_sq in-PSUM -- matching the DMA pace at full clock.
 2. All extra matmul operands are SBUF-resident (x is 0.5 MiB, the
    ysq rows 32 KiB), so the dense PE stream costs no HBM bytes; loads
    stay at ~2.6 MiB vs 33.6 MiB of stores.
Epilogue is a single ScalarE pass (psum + x_sq bias) -> 1 MiB stores.
Startup: the first group's deps are loaded in per-tile pieces so the PE
starts ~5 us earlier; bulk y loads are issued just-in-time behind the
stores in the sync-queue program order so they can't delay the critical
first chunks.
"""

import numpy as np

import concourse.bass as bass
import concourse.mybir as mybir
import concourse.tile as tile
from concourse import bacc
from concourse.bass import ts
from concourse.bass_utils import run_bass_kernel_spmd

N, M, D = 8192, 8192, 128
NCORES = 8
SLAB = N // NCORES  # 1024 rows of x per core
P = 128  # partitions / m-chunk height
MCH = SLAB // P  # 8 m-chunks per core
NT = 512  # matmul free-dim tile (one fp32 PSUM bank)
GW = 4  # n-chunks per PSUM group (4 banks = 8 KiB/partition)
GCOLS = GW * NT  # 2048
NG = M // GCOLS  # 4 column groups

_f32 = mybir.dt.float32
_f16 = mybir.dt.float16
_IDENT = mybir.ActivationFunctionType.Identity

_compiled_nc = None


def _build():
    """Build + compile the single-core Bass program (SPMD across 8 cores)."""
    nc = bacc.Bacc(
        "TRN2",
        target_bir_lowering=False,
        debug=False,
        enable_asserts=False,
        num_devices=NCORES,
    )
    xh = nc.dram_tensor("xh", [D, SLAB], _f16, kind="ExternalInput").ap()
    xl = nc.dram_tensor("xl", [D, SLAB], _f16, kind="ExternalInput").ap()
    yh = nc.dram_tensor("yh", [D, M], _f16, kind="ExternalInput").ap()
    ysq2 = nc.dram_tensor("ysq2", [2, M], _f16, kind="ExternalInput").ap()
    xsq = nc.dram_tensor("xsq", [P, MCH], _f32, kind="ExternalInput").ap()
    dist = nc.dram_tensor("dist", [SLAB, M], _f32, kind="ExternalOutput").ap()

    with tile.TileContext(nc) as tc:
        with (
            tc.tile_pool(name="consts", bufs=1) as cpool,
            tc.tile_pool(name="psum", bufs=2, space="PSUM") as pspool,
            tc.tile_pool(name="obuf", bufs=6) as opool,
        ):
            # Critical-path loads, finest-granularity first: the first
            # matmul needs only yh[:, 0:512] + xh.
            yh_sb = cpool.tile([D, M], _f16)
            xh_sb = cpool.tile([D, SLAB], _f16)
            xl_sb = cpool.tile([D, SLAB], _f16)
            nc.sync.dma_start(yh_sb[:, ts(0, NT)], yh[:, ts(0, NT)])
            nc.sync.dma_start(xh_sb[:], xh[:])
            nc.sync.dma_start(xl_sb[:], xl[:])
            for jj in range(1, GW):
                nc.sync.dma_start(yh_sb[:, ts(jj, NT)], yh[:, ts(jj, NT)])
            ysq_sb = cpool.tile([2, M], _f16)
            nc.sync.dma_start(ysq_sb[:], ysq2[:])
            xsq_sb = cpool.tile([P, MCH], _f32)
            nc.sync.dma_start(xsq_sb[:], xsq[:])
            # ones2[k, p] = 1.0 -- stationary lhsT for the ysq fold matmul.
            ones2 = cpool.tile([2, P], _f16)
            nc.vector.memset(ones2[:], 1.0)

            def emit_block(mc, g):
                """One [128, GCOLS] output block -> dist[mc*128:, g*GCOLS:]."""
                xh_w = xh_sb[:, ts(mc, P)]
                xl_w = xl_sb[:, ts(mc, P)]
                ps = pspool.tile([P, GCOLS], _f32, tag="ps")
                # Dense 12-matmul stream per block; weights held stationary
                # for 4 consecutive matmuls each (xh, xl, ones2).
                for jj in range(GW):
                    nc.tensor.matmul(
                        ps[:, ts(jj, NT)],
                        xh_w,
                        yh_sb[:, ts(g * GW + jj, NT)],
                        start=True,
                        stop=False,
                    )
                for jj in range(GW):
                    nc.tensor.matmul(
                        ps[:, ts(jj, NT)],
                        xl_w,
                        yh_sb[:, ts(g * GW + jj, NT)],
                        start=False,
                        stop=False,
                    )
                for jj in range(GW):
                    nc.tensor.matmul(
                        ps[:, ts(jj, NT)],
                        ones2[:],
                        ysq_sb[:, ts(g * GW + jj, NT)],
                        start=False,
                        stop=True,
                    )
                # Epilogue: out = psum + x_sq (per-partition bias) on ScalarE.
                ot = opool.tile([P, GCOLS], _f32, tag="ot")
                nc.scalar.activation(
                    ot[:],
                    ps[:],
                    _IDENT,
                    bias=xsq_sb[:, mc : mc + 1],
                    scale=1.0,
                )
                nc.sync.dma_start(dist[ts(mc, P), ts(g, GCOLS)], ot[:])
                # Just-in-time bulk loads: next group's yh, one 512-col
                # piece behind each of the first 4 stores of this group.
                # Sits after the store's sem-wait in sync-queue program
                # order, so it can't race ahead of the critical loads.
                if g + 1 < NG and mc < GW:
                    tile_idx = (g + 1) * GW + mc
                    nc.sync.dma_start(
                        yh_sb[:, ts(tile_idx, NT)], yh[:, ts(tile_idx, NT)]
                    )

            for g in range(NG):
                for mc in range(MCH):
                    emit_block(mc, g)

    nc.compile()
    return nc


def _get_nc():
    global _compiled_nc
    if _compiled_nc is None:
        _compiled_nc = _build()
    return _compiled_nc


def make_in_maps(x: np.ndarray, y: np.ndarray) -> list[dict[str, np.ndarray]]:
    x = np.asarray(x, dtype=np.float32)
    y = np.asarray(y, dtype=np.float32)
    x_sq = np.sum(x * x, axis=1, dtype=np.float32)
    y_sq = np.sum(y * y, axis=1, dtype=np.float32)

    xs2t = (-2.0 * x).T  # [D, N] fp32
    xt_hi = xs2t.astype(np.float16)
    xt_lo = (xs2t - xt_hi.astype(np.float32)).astype(np.float16)
    yt_hi = np.ascontiguousarray(y.T.astype(np.float16))  # [D, M]

    ysq_hi = y_sq.astype(np.float16)
    ysq_lo = (y_sq - ysq_hi.astype(np.float32)).astype(np.float16)
    ysq2_in = np.ascontiguousarray(np.stack([ysq_hi, ysq_lo], axis=0))  # [2, M]

    in_maps = []
    for c in range(NCORES):
        sl = slice(c * SLAB, (c + 1) * SLAB)
        # [P, MCH]: column mc holds x_sq for rows mc*128..mc*128+127
        xsq_in = np.ascontiguousarray(x_sq[sl].reshape(MCH, P).T)
        in_maps.append(
            {
                "xh": np.ascontiguousarray(xt_hi[:, sl]),
                "xl": np.ascontiguousarray(xt_lo[:, sl]),
                "yh": yt_hi,
                "ysq2": ysq2_in,
                "xsq": xsq_in,
            }
        )
    return in_maps


def kernel(x: np.ndarray, y: np.ndarray, **run_kwargs) -> np.ndarray:
    nc = _get_nc()
    in_maps = make_in_maps(x, y)
    res = run_bass_kernel_spmd(nc, in_maps, core_ids=list(range(NCORES)), **run_kwargs)
    out = np.concatenate([res.results[c]["dist"] for c in range(NCORES)], axis=0)
    if run_kwargs:
        kernel.last_results = res
    return out
